# revision 1
# baseline (speedup 1.0000x reference)
"""GCLSTM cell (Chebyshev K=3 GCN-gated LSTM) on 8 Trainium2 NeuronCores, v2.

Differences from v1:
  - fp16 data path end to end (PSUM accumulation stays fp32).
  - dma_gather calls merged across tiles up to a chunk budget per call
    (SWDGE ring caps ~128 16-idx DMAs per call), amortizing the ~1us
    per-call prep overhead on the Pool engine.
  - Per-(tile, src-chunk) capacities (static over devices) reduce padding.
  - Sign flip: u = dinv*ew*dinv >= 0, T1 = S(H) with S the positive
    scatter; Tx1 = -T1, Tx2 = 2*S(T1) - H. Signs/factors fold into the
    fused gate weights, so selw = u * onehot(col) is nonnegative and can be
    built on the Activation engine as Relu(u - u*|iota - col|) as well as
    on DVE/Pool via tensor_scalar(is_equal, mult); work is split across
    engines via SELW_PAT.
  - One-hot scatter matmuls in fp16 (1 cycle/row vs 4 for fp32).
  - T1 tiles transposed on PE in phase A, SBUF-resident for the dense
    phase; gate bias applied via a K=1 matmul into PSUM.
"""
import numpy as np

N = 100000
D = 128
NCORES = 8
NPC = N // NCORES            # 12500 nodes per core
TILES = (NPC + 127) // 128   # 98
NPAD = TILES * 128           # 12544
QB = [0, 3968, 7936, 11904, 12500]
QSZ = [QB[i + 1] - QB[i] for i in range(4)]      # 3968,3968,3968,596
NSC = 4
CHUNK_BUDGET = 8             # max 128-slot chunks per dma_gather call
                             # (SWDGE ring: 1024 idxs = 65 descs works,
                             #  1280+ wedges the device)

# engine assignment patterns for selw generation (cycled per chunk).
# Phase A: Act only does 2 copies/tile, so it takes half the selw work.
# Phase C: Act carries 6 activations + 2 copies/tile, so DVE takes 3/4.
SELW_PAT_A = ["v", "a"]
SELW_PAT_C = ["v", "v", "v", "a"]

_CACHE = {}


def _host_prep(X, edge_index, edge_weight, H, C, W, b, conv_W, conv_b):
    row = np.asarray(edge_index[0], dtype=np.int64)
    col = np.asarray(edge_index[1], dtype=np.int64)
    ew = np.asarray(edge_weight, dtype=np.float32)

    deg = np.bincount(row, weights=ew.astype(np.float64), minlength=N)
    deg = deg.astype(np.float32)
    dinv = np.where(deg > 0, deg ** -0.5, 0.0).astype(np.float32)
    u = (dinv[row] * ew * dinv[col])  # positive; w = -u, 2/lambda_max == 1

    dev = col // NPC
    lloc = col % NPC
    tile = lloc // 128
    lsrc = row % NPC
    dsrc = row // NPC
    sc = np.minimum(lsrc // 3968, 3)
    qb = np.array(QB[:4], dtype=np.int64)
    qsz = np.array(QSZ, dtype=np.int64)
    blockrow = dsrc * qsz[sc] + (lsrc - qb[sc])

    counts = np.zeros((NCORES, TILES, NSC), dtype=np.int64)
    np.add.at(counts, (dev, tile, sc), 1)
    caps = np.ceil(counts.max(axis=0) / 128).astype(np.int64)  # [TILES, NSC]

    tile_chunks = caps.sum(axis=1)
    nchunk_tot = int(tile_chunks.sum())
    slots_tot = nchunk_tot * 128
    jbase = np.concatenate([[0], np.cumsum(tile_chunks)[:-1]])

    # greedy gather groups per source chunk s: consecutive tiles while the
    # chunk budget holds. groups_s: list of (t0, t1, nch). Every tile's
    # section must fit one call; caps <= CHUNK_BUDGET each by construction.
    groups = []
    for s in range(NSC):
        gl = []
        t0, acc = 0, 0
        for t in range(TILES):
            c = int(caps[t, s])
            assert c <= CHUNK_BUDGET
            if acc + c > CHUNK_BUDGET:
                gl.append((t0, t, acc))
                t0, acc = t, c
            else:
                acc += c
        gl.append((t0, TILES, acc))
        groups.append(gl)

    # slot layout: for s, for group, for tile in group, for k: 128 slots
    gbase = []          # per s: list of slot bases per group
    tsec = np.zeros((TILES, NSC), dtype=np.int64)  # chunk offset within grp
    tgrp = np.zeros((TILES, NSC), dtype=np.int64)  # group id of tile
    run = 0
    for s in range(NSC):
        bl = []
        for gi, (t0, t1, nch) in enumerate(groups[s]):
            bl.append(run)
            acc = 0
            for t in range(t0, t1):
                tsec[t, s] = acc
                tgrp[t, s] = gi
                acc += caps[t, s]
            run += nch * 128
        gbase.append(bl)
    assert run == slots_tot

    # slot base of chunk k of (t, s)
    def slot_base(t, s, k):
        return gbase[s][tgrp[t, s]] + (tsec[t, s] + k) * 128

    # sort edges by (dev, tile, sc); rank within bucket
    order = np.argsort(dev * (TILES * NSC) + tile * NSC + sc, kind="stable")
    dev_s = dev[order]
    tile_s = tile[order]
    sc_s = sc[order]
    key = dev_s * (TILES * NSC) + tile_s * NSC + sc_s
    idxs = np.arange(len(key))
    same = key[1:] == key[:-1]
    starts = np.concatenate([[0], idxs[1:][~same]])
    runid = np.cumsum(np.concatenate([[0], (~same).astype(np.int64)]))
    rank = idxs - starts[runid]

    sbase_arr = np.zeros((TILES, NSC), dtype=np.int64)
    for t in range(TILES):
        for s in range(NSC):
            sbase_arr[t, s] = slot_base(t, s, 0)
    slot = sbase_arr[tile_s, sc_s] + rank

    idx16 = np.zeros((NCORES, slots_tot), dtype=np.int16)
    colv = np.zeros((NCORES, slots_tot), dtype=np.float32)
    uvv = np.zeros((NCORES, slots_tot), dtype=np.float32)
    idx16[dev_s, slot] = blockrow[order].astype(np.int16)
    colv[dev_s, slot] = (lloc[order] % 128).astype(np.float32)
    uvv[dev_s, slot] = u[order]

    # per-chunk [128 lane, chunk] selw scalars, tile-major s-minor chunk
    # ordering (matmul iteration order)
    chunk_slot_base = np.zeros(nchunk_tot, dtype=np.int64)
    for t in range(TILES):
        jj = jbase[t]
        for s in range(NSC):
            for k in range(caps[t, s]):
                chunk_slot_base[jj] = slot_base(t, s, k)
                jj += 1
    gath = chunk_slot_base[:, None] + np.arange(128)[None, :]

    per_dev = []
    for d in range(NCORES):
        a = idx16[d].reshape(-1, 16).T
        ii = np.tile(a, (8, 1))                       # [128, slots/16]
        colc = colv[d][gath].T.astype(np.float32)     # [128, nchunk_tot]
        uc = uvv[d][gath].T.astype(np.float32)
        per_dev.append((ii, np.ascontiguousarray(colc),
                        np.ascontiguousarray(uc)))

    Wf = np.asarray(W, np.float32)
    CWf = np.asarray(conv_W, np.float32)
    Wb = np.zeros((4, D, 4 * D), dtype=np.float32)
    for g in range(4):
        Wb[0][:, g * D:(g + 1) * D] = Wf[g]
        Wb[1][:, g * D:(g + 1) * D] = CWf[g, 0] - CWf[g, 2]
        Wb[2][:, g * D:(g + 1) * D] = -CWf[g, 1]
        Wb[3][:, g * D:(g + 1) * D] = 2.0 * CWf[g, 2]
    bias = np.concatenate(
        [np.asarray(b, np.float32)[g] + np.asarray(conv_b, np.float32)[g]
         for g in range(4)])

    Xp = np.zeros((NCORES, NPAD, D), np.float16)
    Hp = np.zeros((NCORES, NPAD, D), np.float16)
    Cp = np.zeros((NCORES, NPAD, D), np.float16)
    Xp[:, :NPC] = np.asarray(X, np.float32).reshape(NCORES, NPC, D)
    Hp[:, :NPC] = np.asarray(H, np.float32).reshape(NCORES, NPC, D)
    Cp[:, :NPC] = np.asarray(C, np.float32).reshape(NCORES, NPC, D)
    XT = np.ascontiguousarray(np.transpose(Xp, (0, 2, 1)))  # [NC, D, NPAD]
    HT = np.ascontiguousarray(np.transpose(Hp, (0, 2, 1)))

    Hsh = np.asarray(H, np.float32).astype(np.float16).reshape(NCORES, NPC, D)
    hc = [np.ascontiguousarray(Hsh[:, QB[q]:QB[q + 1], :].reshape(-1, D))
          for q in range(NSC)]

    in_maps = []
    for d in range(NCORES):
        ii, colc, uc = per_dev[d]
        m = {
            "XT": XT[d], "HT": HT[d], "Cp": Cp[d],
            "idx": np.ascontiguousarray(ii),
            "colv": colc, "negcolv": -colc, "uvv": uc, "neguv": -uc,
            "Wb": Wb.reshape(4 * D, 4 * D).astype(np.float16),
            "biasr": bias[None, :].astype(np.float16),
        }
        for s in range(NSC):
            m[f"Hc{s}"] = hc[s]
        in_maps.append(m)

    meta = dict(caps=tuple(map(tuple, caps.tolist())),
                nchunk_tot=nchunk_tot, slots_tot=slots_tot,
                groups=tuple(tuple(g) for g in
                             (tuple(gl) for gl in groups)),
                gbase=tuple(tuple(b) for b in gbase),
                tsec=tuple(map(tuple, tsec.tolist())),
                tgrp=tuple(map(tuple, tgrp.tolist())),
                jbase=tuple(int(x) for x in jbase))
    return in_maps, meta


def _build_program(meta, variant="full", reps=1):
    import concourse.bacc as bacc
    import concourse.tile as tile
    from concourse import mybir
    from concourse.masks import make_identity

    caps = meta["caps"]
    nchunk_tot = meta["nchunk_tot"]
    slots_tot = meta["slots_tot"]
    groups = meta["groups"]
    gbase = meta["gbase"]
    tsec = meta["tsec"]
    tgrp = meta["tgrp"]
    jbase = meta["jbase"]
    f32 = mybir.dt.float32
    f16 = mybir.dt.float16

    ncols_idx = slots_tot // 16

    nc = bacc.Bacc("TRN2", target_bir_lowering=False, debug=False,
                   num_devices=NCORES, num_swdge_queues=4)

    Hc = [nc.dram_tensor(f"Hc{s}", [NCORES * QSZ[s], D], f16,
                         kind="ExternalInput") for s in range(NSC)]
    XTd = nc.dram_tensor("XT", [D, NPAD], f16, kind="ExternalInput")
    HTd = nc.dram_tensor("HT", [D, NPAD], f16, kind="ExternalInput")
    Cpd = nc.dram_tensor("Cp", [NPAD, D], f16, kind="ExternalInput")
    IDX = nc.dram_tensor("idx", [128, ncols_idx], mybir.dt.int16,
                         kind="ExternalInput")
    COL = nc.dram_tensor("colv", [128, nchunk_tot], f32, kind="ExternalInput")
    NCOL = nc.dram_tensor("negcolv", [128, nchunk_tot], f32,
                          kind="ExternalInput")
    UV = nc.dram_tensor("uvv", [128, nchunk_tot], f32, kind="ExternalInput")
    NUV = nc.dram_tensor("neguv", [128, nchunk_tot], f32,
                         kind="ExternalInput")
    WB = nc.dram_tensor("Wb", [4 * D, 4 * D], f16, kind="ExternalInput")
    BIASR = nc.dram_tensor("biasr", [1, 4 * D], f16, kind="ExternalInput")
    OUT = nc.dram_tensor("OUT", [NPAD, D], f16, kind="ExternalOutput")

    cc_in = [nc.dram_tensor(f"cc_in{q}", [QSZ[q], D], f16)
             for q in range(NSC)]
    cc_out = [nc.dram_tensor(f"cc_out{q}", [NCORES * QSZ[q], D], f16,
                             addr_space="Shared") for q in range(NSC)]

    qn = [0]

    def next_q():
        q = qn[0] % 4
        qn[0] += 1
        return q

    pat_i = [0]
    pat_cur = [SELW_PAT_A]

    def next_eng():
        p = pat_cur[0]
        e = p[pat_i[0] % len(p)]
        pat_i[0] += 1
        return e

    with tile.TileContext(nc) as tc:
        import contextlib
        ctx = contextlib.ExitStack()
        with ctx:
            AF = mybir.ActivationFunctionType
            ALU = mybir.AluOpType

            const = ctx.enter_context(tc.tile_pool(name="const", bufs=1))
            gp = [ctx.enter_context(
                tc.tile_pool(name=f"g{s}", bufs=5)) for s in range(NSC)]
            sp = ctx.enter_context(tc.tile_pool(name="selw", bufs=12))
            ldp = ctx.enter_context(tc.tile_pool(name="ld", bufs=12))
            outp = ctx.enter_context(tc.tile_pool(name="outp", bufs=18))
            ps_a = ctx.enter_context(tc.tile_pool(name="ps_a", bufs=4,
                                                  space="PSUM"))
            ps_t = ctx.enter_context(tc.tile_pool(name="ps_t", bufs=2,
                                                  space="PSUM"))
            ps_g = ctx.enter_context(tc.tile_pool(name="ps_g", bufs=2,
                                                  space="PSUM"))

            # --- resident constants ---------------------------------------
            idx_sb = const.tile([128, ncols_idx], mybir.dt.int16)
            nc.sync.dma_start(out=idx_sb[:], in_=IDX[:])
            col_sb = const.tile([128, nchunk_tot], f32)
            nc.sync.dma_start(out=col_sb[:], in_=COL[:])
            ncol_sb = const.tile([128, nchunk_tot], f32)
            nc.sync.dma_start(out=ncol_sb[:], in_=NCOL[:])
            u_sb = const.tile([128, nchunk_tot], f32)
            nc.sync.dma_start(out=u_sb[:], in_=UV[:])
            nu_sb = const.tile([128, nchunk_tot], f32)
            nc.sync.dma_start(out=nu_sb[:], in_=NUV[:])
            wb_sb = [const.tile([128, 4 * D], f16, tag=f"wb{i}",
                                name=f"wb{i}") for i in range(4)]
            for i in range(4):
                nc.sync.dma_start(out=wb_sb[i][:],
                                  in_=WB[i * 128:(i + 1) * 128, :])
            biasr_sb = const.tile([1, 4 * D], f16)
            nc.sync.dma_start(out=biasr_sb[:], in_=BIASR[:])
            ones1 = const.tile([1, 128], f16)
            nc.vector.memset(ones1[:], 1.0)

            identf = const.tile([128, 128], f32)
            make_identity(nc, identf[:])
            ident16 = const.tile([128, 128], f16)
            nc.vector.tensor_copy(out=ident16[:], in_=identf[:])
            iota_i = const.tile([128, 128], mybir.dt.int32)
            nc.gpsimd.iota(iota_i[:], pattern=[[1, 128]], base=0,
                           channel_multiplier=0)
            iota_h = const.tile([128, 128], f16)
            nc.vector.tensor_copy(out=iota_h[:], in_=iota_i[:])

            t1T_all = const.tile([128, TILES * 128], f16)

            def make_selw(j):
                e = next_eng()
                selw = sp.tile([128, 128], f16, tag="selw")
                if e == "v":
                    nc.vector.tensor_scalar(
                        out=selw[:], in0=iota_h[:],
                        scalar1=col_sb[:, j:j + 1],
                        scalar2=u_sb[:, j:j + 1],
                        op0=ALU.is_equal, op1=ALU.mult)
                elif e == "p":
                    nc.gpsimd.tensor_scalar(
                        out=selw[:], in0=iota_h[:],
                        scalar1=col_sb[:, j:j + 1],
                        scalar2=u_sb[:, j:j + 1],
                        op0=ALU.is_equal, op1=ALU.mult)
                else:
                    tmp = sp.tile([128, 128], f16, tag="selt")
                    nc.scalar.activation(out=tmp[:], in_=iota_h[:],
                                         func=AF.Abs,
                                         bias=ncol_sb[:, j:j + 1],
                                         scale=1.0)
                    nc.scalar.activation(out=selw[:], in_=tmp[:],
                                         func=AF.Relu,
                                         bias=u_sb[:, j:j + 1],
                                         scale=nu_sb[:, j:j + 1])
                return selw

            # gather state: per s, tiles of current group
            def maybe_gather(src_tensors, t, gcur):
                for s in range(NSC):
                    gi = tgrp[t][s]
                    if gcur[s][0] == gi:
                        continue
                    t0, t1, nch = groups[s][gi]
                    if nch == 0:
                        gcur[s] = (gi, None)
                        continue
                    gt = gp[s].tile([128, CHUNK_BUDGET, 128], f16,
                                    tag=f"g{s}")
                    base = gbase[s][gi]
                    ni = nch * 128
                    nc.gpsimd.dma_gather(
                        out_ap=gt[:, :nch, :],
                        in_ap=src_tensors[s][:],
                        idxs_ap=idx_sb[:, base // 16:(base + ni) // 16],
                        num_idxs=ni,
                        num_idxs_reg=ni,
                        elem_size=D,
                        queue_num=next_q(),
                    )
                    gcur[s] = (gi, gt)

            def scatter_tile(t, gcur, transposed):
                ps = ps_a.tile([128, 128], f32, tag="scat")
                nch_t = sum(caps[t])
                ch = 0
                j0 = jbase[t]
                for s in range(NSC):
                    for k in range(caps[t][s]):
                        selw = make_selw(j0 + ch)
                        gsl = gcur[s][1][:, tsec[t][s] + k, :]
                        if transposed:
                            nc.tensor.matmul(ps[:], lhsT=gsl, rhs=selw[:],
                                             start=(ch == 0),
                                             stop=(ch == nch_t - 1))
                        else:
                            nc.tensor.matmul(ps[:], lhsT=selw[:], rhs=gsl,
                                             start=(ch == 0),
                                             stop=(ch == nch_t - 1))
                        ch += 1
                return ps

            # --- phase A: T1 = S(H); keep transposed tiles resident -------
            def phase_a(iv=None):
                pat_cur[0] = SELW_PAT_A
                gcur = [(-1, None)] * NSC
                for t in range(TILES):
                    maybe_gather(Hc, t, gcur)
                    ps = scatter_tile(t, gcur, transposed=False)
                    tx1 = outp.tile([128, 128], f16, tag="tx1")
                    nc.scalar.activation(out=tx1[:], in_=ps[:], func=AF.Copy)
                    rows = min(128, NPC - t * 128)
                    q = min((t * 128) // 3968, 3)
                    off = t * 128 - QB[q]
                    nc.sync.dma_start(out=cc_in[q][off:off + rows, :],
                                      in_=tx1[:rows, :])
                    pst = ps_t.tile([128, 128], f16, tag="tr")
                    nc.tensor.transpose(out=pst[:], in_=tx1[:],
                                        identity=ident16[:])
                    nc.vector.tensor_copy(
                        out=t1T_all[:, t * 128:(t + 1) * 128], in_=pst[:])

            def phase_b():
                for q in range(NSC):
                    nc.gpsimd.collective_compute(
                        "AllGather",
                        mybir.AluOpType.bypass,
                        replica_groups=[list(range(NCORES))],
                        ins=[cc_in[q][:]],
                        outs=[cc_out[q][:]],
                    )

            # --- phase C: S2 = S(T1) transposed + dense + LSTM ------------
            def phase_c(iv=None):
                pat_cur[0] = SELW_PAT_C
                gcur = [(-1, None)] * NSC
                for t in range(TILES):
                    maybe_gather(cc_out, t, gcur)
                    ps2 = scatter_tile(t, gcur, transposed=True)
                    t2T = outp.tile([128, 128], f16, tag="t2T")
                    nc.scalar.activation(out=t2T[:], in_=ps2[:], func=AF.Copy)

                    xT = ldp.tile([128, 128], f16, tag="xT")
                    nc.sync.dma_start(out=xT[:],
                                      in_=XTd[:, t * 128:(t + 1) * 128])
                    hT = ldp.tile([128, 128], f16, tag="hT")
                    nc.sync.dma_start(out=hT[:],
                                      in_=HTd[:, t * 128:(t + 1) * 128])
                    ct = ldp.tile([128, 128], f16, tag="ct")
                    nc.sync.dma_start(out=ct[:],
                                      in_=Cpd[t * 128:(t + 1) * 128, :])

                    gps = ps_g.tile([128, 4 * D], f32, tag="G")
                    nc.tensor.matmul(gps[:], lhsT=ones1[:], rhs=biasr_sb[:],
                                     start=True, stop=False)
                    nc.tensor.matmul(gps[:], lhsT=xT[:], rhs=wb_sb[0][:],
                                     start=False, stop=False)
                    nc.tensor.matmul(gps[:], lhsT=hT[:], rhs=wb_sb[1][:],
                                     start=False, stop=False)
                    nc.tensor.matmul(gps[:],
                                     lhsT=t1T_all[:, t * 128:(t + 1) * 128],
                                     rhs=wb_sb[2][:], start=False, stop=False)
                    nc.tensor.matmul(gps[:], lhsT=t2T[:], rhs=wb_sb[3][:],
                                     start=False, stop=True)

                    act = outp.tile([128, 4 * D], f16, tag="act")
                    nc.scalar.activation(out=act[:, 0:128], in_=gps[:, 0:128],
                                         func=AF.Sigmoid)
                    nc.scalar.activation(out=act[:, 128:256],
                                         in_=gps[:, 128:256], func=AF.Sigmoid)
                    nc.scalar.activation(out=act[:, 256:384],
                                         in_=gps[:, 256:384], func=AF.Tanh)
                    nc.scalar.activation(out=act[:, 384:512],
                                         in_=gps[:, 384:512], func=AF.Sigmoid)

                    it = outp.tile([128, 128], f16, tag="it")
                    nc.vector.tensor_tensor(out=it[:], in0=act[:, 0:128],
                                            in1=act[:, 256:384], op=ALU.mult)
                    fc = outp.tile([128, 128], f16, tag="fc")
                    nc.gpsimd.tensor_tensor(out=fc[:], in0=act[:, 128:256],
                                            in1=ct[:], op=ALU.mult)
                    cn = outp.tile([128, 128], f16, tag="cn")
                    nc.vector.tensor_tensor(out=cn[:], in0=fc[:], in1=it[:],
                                            op=ALU.add)
                    tct = outp.tile([128, 128], f16, tag="tct")
                    nc.scalar.activation(out=tct[:], in_=cn[:], func=AF.Tanh)
                    hn = outp.tile([128, 128], f16, tag="hn")
                    nc.vector.tensor_tensor(out=hn[:], in0=act[:, 384:512],
                                            in1=tct[:], op=ALU.mult)
                    nc.sync.dma_start(out=OUT[t * 128:(t + 1) * 128, :],
                                      in_=hn[:])

            if variant == "full":
                if reps == 1:
                    phase_a()
                    phase_b()
                    phase_c()
                else:
                    for _ in range(reps):
                        phase_a()
                        phase_b()
                        phase_c()
            elif variant == "a_only":
                tc.For_i_unrolled(0, reps, 1, phase_a, max_unroll=1)
            elif variant == "c_only":
                tc.For_i_unrolled(0, reps, 1, phase_c, max_unroll=1)
            else:
                raise ValueError(variant)

    nc.compile()
    return nc


def kernel(X, edge_index, edge_weight, H, C, W, b, conv_W, conv_b):
    from concourse.bass_utils import run_bass_kernel_spmd

    in_maps, meta = _host_prep(X, edge_index, edge_weight, H, C, W, b,
                               conv_W, conv_b)
    key = (meta["caps"], meta["groups"])
    if key not in _CACHE:
        _CACHE[key] = _build_program(meta)
    nc = _CACHE[key]

    res = run_bass_kernel_spmd(nc, in_maps, list(range(NCORES)))
    out = np.empty((N, D), np.float32)
    for d in range(NCORES):
        out[d * NPC:(d + 1) * NPC] = res.results[d]["OUT"][:NPC].astype(
            np.float32)
    return out



# revision 20
# speedup vs baseline: 1.7200x; 1.7200x over previous
"""GCLSTM cell (Chebyshev K=3 GCN-gated LSTM) on 8 Trainium2 NeuronCores, v2.

Differences from v1:
  - fp16 data path end to end (PSUM accumulation stays fp32).
  - dma_gather calls merged across tiles up to a chunk budget per call
    (SWDGE ring caps ~128 16-idx DMAs per call), amortizing the ~1us
    per-call prep overhead on the Pool engine.
  - Per-(tile, src-chunk) capacities (static over devices) reduce padding.
  - Sign flip: u = dinv*ew*dinv >= 0, T1 = S(H) with S the positive
    scatter; Tx1 = -T1, Tx2 = 2*S(T1) - H. Signs/factors fold into the
    fused gate weights, so selw = u * onehot(col) is nonnegative and can be
    built on the Activation engine as Relu(u - u*|iota - col|) as well as
    on DVE/Pool via tensor_scalar(is_equal, mult); work is split across
    engines via SELW_PAT.
  - One-hot scatter matmuls in fp16 (1 cycle/row vs 4 for fp32).
  - T1 tiles transposed on PE in phase A, SBUF-resident for the dense
    phase; gate bias applied via a K=1 matmul into PSUM.
"""
import numpy as np

N = 100000
D = 128
NCORES = 8
NPC = N // NCORES            # 12500 nodes per core
TILES = (NPC + 127) // 128   # 98
NPAD = TILES * 128           # 12544
QB = [0, 3968, 7936, 11904, 12500]
QSZ = [QB[i + 1] - QB[i] for i in range(4)]      # 3968,3968,3968,596
NSC = 4
CHUNK_BUDGET = 8             # max 128-slot chunks per dma_gather call
                             # (SWDGE ring: 1024 idxs = 65 descs works,
                             #  1280+ wedges the device)

# engine assignment patterns for selw generation (cycled per chunk).
# Phase A: Act only does 2 copies/tile, so it takes half the selw work.
# Phase C: Act carries 6 activations + 2 copies/tile, so DVE takes 3/4.
SELW_PAT_A = ["v", "a"]
SELW_PAT_C = ["v", "v", "v", "a"]

_CACHE = {}


def _host_prep(X, edge_index, edge_weight, H, C, W, b, conv_W, conv_b):
    row = np.asarray(edge_index[0], dtype=np.int64)
    col = np.asarray(edge_index[1], dtype=np.int64)
    ew = np.asarray(edge_weight, dtype=np.float32)

    deg = np.bincount(row, weights=ew.astype(np.float64), minlength=N)
    deg = deg.astype(np.float32)
    dinv = np.where(deg > 0, deg ** -0.5, 0.0).astype(np.float32)
    u = (dinv[row] * ew * dinv[col])  # positive; w = -u, 2/lambda_max == 1

    dev = col // NPC
    lloc = col % NPC
    tile = lloc // 128
    lsrc = row % NPC
    dsrc = row // NPC
    sc = np.minimum(lsrc // 3968, 3)
    qb = np.array(QB[:4], dtype=np.int64)
    qsz = np.array(QSZ, dtype=np.int64)
    blockrow = dsrc * qsz[sc] + (lsrc - qb[sc])

    counts = np.zeros((NCORES, TILES, NSC), dtype=np.int64)
    np.add.at(counts, (dev, tile, sc), 1)
    caps = np.ceil(counts.max(axis=0) / 128).astype(np.int64)  # [TILES, NSC]

    tile_chunks = caps.sum(axis=1)
    nchunk_tot = int(tile_chunks.sum())
    slots_tot = nchunk_tot * 128
    jbase = np.concatenate([[0], np.cumsum(tile_chunks)[:-1]])

    # greedy gather groups per source chunk s: consecutive tiles while the
    # chunk budget holds. groups_s: list of (t0, t1, nch). Every tile's
    # section must fit one call; caps <= CHUNK_BUDGET each by construction.
    groups = []
    for s in range(NSC):
        gl = []
        t0, acc = 0, 0
        for t in range(TILES):
            c = int(caps[t, s])
            assert c <= CHUNK_BUDGET
            if acc + c > CHUNK_BUDGET:
                gl.append((t0, t, acc))
                t0, acc = t, c
            else:
                acc += c
        gl.append((t0, TILES, acc))
        groups.append(gl)

    # slot layout: for s, for group, for tile in group, for k: 128 slots
    gbase = []          # per s: list of slot bases per group
    tsec = np.zeros((TILES, NSC), dtype=np.int64)  # chunk offset within grp
    tgrp = np.zeros((TILES, NSC), dtype=np.int64)  # group id of tile
    run = 0
    for s in range(NSC):
        bl = []
        for gi, (t0, t1, nch) in enumerate(groups[s]):
            bl.append(run)
            acc = 0
            for t in range(t0, t1):
                tsec[t, s] = acc
                tgrp[t, s] = gi
                acc += caps[t, s]
            run += nch * 128
        gbase.append(bl)
    assert run == slots_tot

    # slot base of chunk k of (t, s)
    def slot_base(t, s, k):
        return gbase[s][tgrp[t, s]] + (tsec[t, s] + k) * 128

    # sort edges by (dev, tile, sc); rank within bucket
    order = np.argsort(dev * (TILES * NSC) + tile * NSC + sc, kind="stable")
    dev_s = dev[order]
    tile_s = tile[order]
    sc_s = sc[order]
    key = dev_s * (TILES * NSC) + tile_s * NSC + sc_s
    idxs = np.arange(len(key))
    same = key[1:] == key[:-1]
    starts = np.concatenate([[0], idxs[1:][~same]])
    runid = np.cumsum(np.concatenate([[0], (~same).astype(np.int64)]))
    rank = idxs - starts[runid]

    sbase_arr = np.zeros((TILES, NSC), dtype=np.int64)
    for t in range(TILES):
        for s in range(NSC):
            sbase_arr[t, s] = slot_base(t, s, 0)
    slot = sbase_arr[tile_s, sc_s] + rank

    idx16 = np.zeros((NCORES, slots_tot), dtype=np.int16)
    colv = np.zeros((NCORES, slots_tot), dtype=np.float32)
    uvv = np.zeros((NCORES, slots_tot), dtype=np.float32)
    idx16[dev_s, slot] = blockrow[order].astype(np.int16)
    colv[dev_s, slot] = (lloc[order] % 128).astype(np.float32)
    uvv[dev_s, slot] = u[order]

    # per-chunk [128 lane, chunk] selw scalars, tile-major s-minor chunk
    # ordering (matmul iteration order)
    chunk_slot_base = np.zeros(nchunk_tot, dtype=np.int64)
    for t in range(TILES):
        jj = jbase[t]
        for s in range(NSC):
            for k in range(caps[t, s]):
                chunk_slot_base[jj] = slot_base(t, s, k)
                jj += 1
    gath = chunk_slot_base[:, None] + np.arange(128)[None, :]

    per_dev = []
    for d in range(NCORES):
        a = idx16[d].reshape(-1, 16).T
        ii = np.tile(a, (8, 1))                       # [128, slots/16]
        colc = colv[d][gath].T.astype(np.float32)     # [128, nchunk_tot]
        uc = uvv[d][gath].T.astype(np.float32)
        per_dev.append((ii, np.ascontiguousarray(colc),
                        np.ascontiguousarray(uc)))

    Wf = np.asarray(W, np.float32)
    CWf = np.asarray(conv_W, np.float32)
    # gate column order [i, f, o, c]: sigmoid gates contiguous in 0:3D,
    # tanh gate in 3D:4D, so activations batch into two instructions.
    GORD = [0, 1, 3, 2]
    Wb = np.zeros((4, D, 4 * D), dtype=np.float32)
    for j, g in enumerate(GORD):
        Wb[0][:, j * D:(j + 1) * D] = Wf[g]
        Wb[1][:, j * D:(j + 1) * D] = CWf[g, 0] - CWf[g, 2]
        Wb[2][:, j * D:(j + 1) * D] = -CWf[g, 1]
        Wb[3][:, j * D:(j + 1) * D] = 2.0 * CWf[g, 2]
    bias = np.concatenate(
        [np.asarray(b, np.float32)[g] + np.asarray(conv_b, np.float32)[g]
         for g in GORD])

    Xp = np.zeros((NCORES, NPAD, D), np.float16)
    Hp = np.zeros((NCORES, NPAD, D), np.float16)
    Cp = np.zeros((NCORES, NPAD, D), np.float16)
    Xp[:, :NPC] = np.asarray(X, np.float32).reshape(NCORES, NPC, D)
    Hp[:, :NPC] = np.asarray(H, np.float32).reshape(NCORES, NPC, D)
    Cp[:, :NPC] = np.asarray(C, np.float32).reshape(NCORES, NPC, D)
    XT = np.ascontiguousarray(np.transpose(Xp, (0, 2, 1)))  # [NC, D, NPAD]
    HT = np.ascontiguousarray(np.transpose(Hp, (0, 2, 1)))

    Hsh = np.asarray(H, np.float32).astype(np.float16).reshape(NCORES, NPC, D)
    hc = [np.ascontiguousarray(Hsh[:, QB[q]:QB[q + 1], :].reshape(-1, D))
          for q in range(NSC)]

    in_maps = []
    for d in range(NCORES):
        ii, colc, uc = per_dev[d]
        m = {
            "XT": XT[d], "HT": HT[d], "Cp": Cp[d],
            "idx": np.ascontiguousarray(ii),
            "colv": colc, "negcolv": -colc, "uvv": uc, "neguv": -uc,
            "Wb": Wb.reshape(4 * D, 4 * D).astype(np.float16),
            "biasr": bias[None, :].astype(np.float16),
        }
        for s in range(NSC):
            m[f"Hc{s}"] = hc[s]
        in_maps.append(m)

    meta = dict(caps=tuple(map(tuple, caps.tolist())),
                nchunk_tot=nchunk_tot, slots_tot=slots_tot,
                groups=tuple(tuple(g) for g in
                             (tuple(gl) for gl in groups)),
                gbase=tuple(tuple(b) for b in gbase),
                tsec=tuple(map(tuple, tsec.tolist())),
                tgrp=tuple(map(tuple, tgrp.tolist())),
                jbase=tuple(int(x) for x in jbase))
    return in_maps, meta


def _build_program(meta, variant="full", reps=1):
    import concourse.bacc as bacc
    import concourse.tile as tile
    from concourse import mybir
    from concourse.masks import make_identity

    caps = meta["caps"]
    nchunk_tot = meta["nchunk_tot"]
    slots_tot = meta["slots_tot"]
    groups = meta["groups"]
    gbase = meta["gbase"]
    tsec = meta["tsec"]
    tgrp = meta["tgrp"]
    jbase = meta["jbase"]
    f32 = mybir.dt.float32
    f16 = mybir.dt.float16

    ncols_idx = slots_tot // 16

    nc = bacc.Bacc("TRN2", target_bir_lowering=False, debug=False,
                   num_devices=NCORES, num_swdge_queues=4)

    Hc = [nc.dram_tensor(f"Hc{s}", [NCORES * QSZ[s], D], f16,
                         kind="ExternalInput") for s in range(NSC)]
    XTd = nc.dram_tensor("XT", [D, NPAD], f16, kind="ExternalInput")
    HTd = nc.dram_tensor("HT", [D, NPAD], f16, kind="ExternalInput")
    Cpd = nc.dram_tensor("Cp", [NPAD, D], f16, kind="ExternalInput")
    IDX = nc.dram_tensor("idx", [128, ncols_idx], mybir.dt.int16,
                         kind="ExternalInput")
    COL = nc.dram_tensor("colv", [128, nchunk_tot], f32, kind="ExternalInput")
    NCOL = nc.dram_tensor("negcolv", [128, nchunk_tot], f32,
                          kind="ExternalInput")
    UV = nc.dram_tensor("uvv", [128, nchunk_tot], f32, kind="ExternalInput")
    NUV = nc.dram_tensor("neguv", [128, nchunk_tot], f32,
                         kind="ExternalInput")
    WB = nc.dram_tensor("Wb", [4 * D, 4 * D], f16, kind="ExternalInput")
    BIASR = nc.dram_tensor("biasr", [1, 4 * D], f16, kind="ExternalInput")
    OUT = nc.dram_tensor("OUT", [NPAD, D], f16, kind="ExternalOutput")

    cc_in = [nc.dram_tensor(f"cc_in{q}", [QSZ[q], D], f16)
             for q in range(NSC)]
    cc_out = [nc.dram_tensor(f"cc_out{q}", [NCORES * QSZ[q], D], f16,
                             addr_space="Shared") for q in range(NSC)]

    qn = [0]

    def next_q():
        q = qn[0] % 4
        qn[0] += 1
        return q

    pat_i = [0]
    pat_cur = [SELW_PAT_A]

    def next_eng():
        p = pat_cur[0]
        e = p[pat_i[0] % len(p)]
        pat_i[0] += 1
        return e

    with tile.TileContext(nc) as tc:
        import contextlib
        ctx = contextlib.ExitStack()
        with ctx:
            AF = mybir.ActivationFunctionType
            ALU = mybir.AluOpType

            const = ctx.enter_context(tc.tile_pool(name="const", bufs=1))
            gp = [ctx.enter_context(
                tc.tile_pool(name=f"g{s}", bufs=5)) for s in range(NSC)]
            sp = ctx.enter_context(tc.tile_pool(name="selw", bufs=12))
            ldp = ctx.enter_context(tc.tile_pool(name="ld", bufs=12))
            outp = ctx.enter_context(tc.tile_pool(name="outp", bufs=18))
            ps_a = ctx.enter_context(tc.tile_pool(name="ps_a", bufs=4,
                                                  space="PSUM"))

            # --- resident constants ---------------------------------------
            idx_sb = const.tile([128, ncols_idx], mybir.dt.int16)
            nc.sync.dma_start(out=idx_sb[:], in_=IDX[:])
            col_sb = const.tile([128, nchunk_tot], f32)
            nc.sync.dma_start(out=col_sb[:], in_=COL[:])
            ncol_sb = const.tile([128, nchunk_tot], f32)
            nc.sync.dma_start(out=ncol_sb[:], in_=NCOL[:])
            u_sb = const.tile([128, nchunk_tot], f32)
            nc.sync.dma_start(out=u_sb[:], in_=UV[:])
            nu_sb = const.tile([128, nchunk_tot], f32)
            nc.sync.dma_start(out=nu_sb[:], in_=NUV[:])
            wb_sb = [const.tile([128, 4 * D], f16, tag=f"wb{i}",
                                name=f"wb{i}") for i in range(4)]
            for i in range(4):
                nc.sync.dma_start(out=wb_sb[i][:],
                                  in_=WB[i * 128:(i + 1) * 128, :])
            biasr_sb = const.tile([1, 4 * D], f16)
            nc.sync.dma_start(out=biasr_sb[:], in_=BIASR[:])
            ones1 = const.tile([1, 128], f16)
            nc.vector.memset(ones1[:], 1.0)

            identf = const.tile([128, 128], f32)
            make_identity(nc, identf[:])
            ident16 = const.tile([128, 128], f16)
            nc.vector.tensor_copy(out=ident16[:], in_=identf[:])
            iota_i = const.tile([128, 128], mybir.dt.int32)
            nc.gpsimd.iota(iota_i[:], pattern=[[1, 128]], base=0,
                           channel_multiplier=0)
            iota_h = const.tile([128, 128], f16)
            nc.vector.tensor_copy(out=iota_h[:], in_=iota_i[:])

            t1T_all = const.tile([128, TILES * 128], f16)

            def make_selw(j):
                e = next_eng()
                selw = sp.tile([128, 128], f16, tag="selw")
                if e == "v":
                    nc.vector.tensor_scalar(
                        out=selw[:], in0=iota_h[:],
                        scalar1=col_sb[:, j:j + 1],
                        scalar2=u_sb[:, j:j + 1],
                        op0=ALU.is_equal, op1=ALU.mult)
                elif e == "p":
                    nc.gpsimd.tensor_scalar(
                        out=selw[:], in0=iota_h[:],
                        scalar1=col_sb[:, j:j + 1],
                        scalar2=u_sb[:, j:j + 1],
                        op0=ALU.is_equal, op1=ALU.mult)
                else:
                    tmp = sp.tile([128, 128], f16, tag="selt")
                    nc.scalar.activation(out=tmp[:], in_=iota_h[:],
                                         func=AF.Abs,
                                         bias=ncol_sb[:, j:j + 1],
                                         scale=1.0)
                    nc.scalar.activation(out=selw[:], in_=tmp[:],
                                         func=AF.Relu,
                                         bias=u_sb[:, j:j + 1],
                                         scale=nu_sb[:, j:j + 1])
                return selw

            # gather state: per s, tiles of current group
            def maybe_gather(src_tensors, t, gcur):
                for s in range(NSC):
                    gi = tgrp[t][s]
                    if gcur[s][0] == gi:
                        continue
                    t0, t1, nch = groups[s][gi]
                    if nch == 0:
                        gcur[s] = (gi, None)
                        continue
                    gt = gp[s].tile([128, CHUNK_BUDGET, 128], f16,
                                    tag=f"g{s}")
                    base = gbase[s][gi]
                    ni = nch * 128
                    nc.gpsimd.dma_gather(
                        out_ap=gt[:, :nch, :],
                        in_ap=src_tensors[s][:],
                        idxs_ap=idx_sb[:, base // 16:(base + ni) // 16],
                        num_idxs=ni,
                        num_idxs_reg=ni,
                        elem_size=D,
                        queue_num=next_q(),
                    )
                    gcur[s] = (gi, gt)

            def scatter_tile(t, gcur, transposed):
                ps = ps_a.tile([128, 128], f32, tag="scat")
                nch_t = sum(caps[t])
                ch = 0
                j0 = jbase[t]
                for s in range(NSC):
                    for k in range(caps[t][s]):
                        selw = make_selw(j0 + ch)
                        gsl = gcur[s][1][:, tsec[t][s] + k, :]
                        if transposed:
                            nc.tensor.matmul(ps[:], lhsT=gsl, rhs=selw[:],
                                             start=(ch == 0),
                                             stop=(ch == nch_t - 1))
                        else:
                            nc.tensor.matmul(ps[:], lhsT=selw[:], rhs=gsl,
                                             start=(ch == 0),
                                             stop=(ch == nch_t - 1))
                        ch += 1
                return ps

            # --- phase A: T1 = S(H); keep transposed tiles resident -------
            def phase_a(iv=None):
                pat_cur[0] = SELW_PAT_A
                gcur = [(-1, None)] * NSC
                with tc.tile_pool(name="ps_t", bufs=4, space="PSUM") as ps_t:
                    for t in range(TILES):
                        maybe_gather(Hc, t, gcur)
                        ps = scatter_tile(t, gcur, transposed=False)
                        tx1 = outp.tile([128, 128], f16, tag="tx1")
                        nc.scalar.activation(out=tx1[:], in_=ps[:],
                                             func=AF.Copy)
                        rows = min(128, NPC - t * 128)
                        q = min((t * 128) // 3968, 3)
                        off = t * 128 - QB[q]
                        nc.sync.dma_start(out=cc_in[q][off:off + rows, :],
                                          in_=tx1[:rows, :])
                        pst = ps_t.tile([128, 128], f16, tag="tr")
                        nc.tensor.transpose(out=pst[:], in_=tx1[:],
                                            identity=ident16[:])
                        nc.vector.tensor_copy(
                            out=t1T_all[:, t * 128:(t + 1) * 128], in_=pst[:])

            def phase_b():
                for q in range(NSC):
                    nc.gpsimd.collective_compute(
                        "AllGather",
                        mybir.AluOpType.bypass,
                        replica_groups=[list(range(NCORES))],
                        ins=[cc_in[q][:]],
                        outs=[cc_out[q][:]],
                    )

            # --- phase C: S2 = S(T1) transposed + dense + LSTM ------------
            def phase_c(iv=None, src=None, lvl=3, acts=True, ew=True,
                        pooltt=False):
                if src is None:
                    src = cc_out
                pat_cur[0] = SELW_PAT_C
                gcur = [(-1, None)] * NSC
                with tc.tile_pool(name="ps_g", bufs=4, space="PSUM") as ps_g:
                    phase_c_body(src, lvl, acts, ew, pooltt, gcur, ps_g)

            def phase_c_body(src, lvl, acts, ew, pooltt, gcur, ps_g):
                for t in range(TILES):
                    maybe_gather(src, t, gcur)
                    ps2 = scatter_tile(t, gcur, transposed=True)
                    t2T = outp.tile([128, 128], f16, tag="t2T")
                    nc.scalar.activation(out=t2T[:], in_=ps2[:], func=AF.Copy)
                    if lvl < 2:
                        nc.sync.dma_start(out=OUT[t * 128:(t + 1) * 128, :],
                                          in_=t2T[:])
                        continue

                    xT = ldp.tile([128, 128], f16, tag="xT")
                    nc.sync.dma_start(out=xT[:],
                                      in_=XTd[:, t * 128:(t + 1) * 128])
                    hT = ldp.tile([128, 128], f16, tag="hT")
                    nc.sync.dma_start(out=hT[:],
                                      in_=HTd[:, t * 128:(t + 1) * 128])
                    ct = ldp.tile([128, 128], f16, tag="ct")
                    nc.sync.dma_start(out=ct[:],
                                      in_=Cpd[t * 128:(t + 1) * 128, :])

                    gps = ps_g.tile([128, 4 * D], f32, tag="G")
                    nc.tensor.matmul(gps[:], lhsT=ones1[:], rhs=biasr_sb[:],
                                     start=True, stop=False)
                    nc.tensor.matmul(gps[:], lhsT=xT[:], rhs=wb_sb[0][:],
                                     start=False, stop=False)
                    nc.tensor.matmul(gps[:], lhsT=hT[:], rhs=wb_sb[1][:],
                                     start=False, stop=False)
                    nc.tensor.matmul(gps[:],
                                     lhsT=t1T_all[:, t * 128:(t + 1) * 128],
                                     rhs=wb_sb[2][:], start=False, stop=False)
                    nc.tensor.matmul(gps[:], lhsT=t2T[:], rhs=wb_sb[3][:],
                                     start=False, stop=True)
                    if lvl < 3:
                        gout = outp.tile([128, 128], f16, tag="gout")
                        nc.vector.tensor_copy(out=gout[:], in_=gps[:, 0:128])
                        nc.sync.dma_start(out=OUT[t * 128:(t + 1) * 128, :],
                                          in_=gout[:])
                        continue

                    AFS = AF.Sigmoid if acts else AF.Copy
                    AFT = AF.Tanh if acts else AF.Copy
                    act = outp.tile([128, 4 * D], f16, tag="act")
                    nc.scalar.activation(out=act[:, 0:384], in_=gps[:, 0:384],
                                         func=AFS)
                    nc.scalar.activation(out=act[:, 384:512],
                                         in_=gps[:, 384:512], func=AFT)
                    if not ew:
                        nc.sync.dma_start(out=OUT[t * 128:(t + 1) * 128, :],
                                          in_=act[:, 0:128])
                        continue

                    # gate cols: I 0:128, F 128:256, O 256:384, T 384:512
                    it = outp.tile([128, 128], f16, tag="it")
                    nc.vector.tensor_tensor(out=it[:], in0=act[:, 0:128],
                                            in1=act[:, 384:512], op=ALU.mult)
                    fc = outp.tile([128, 128], f16, tag="fc")
                    if pooltt:
                        nc.gpsimd.tensor_tensor(out=fc[:], in0=act[:, 128:256],
                                                in1=ct[:], op=ALU.mult)
                    else:
                        nc.vector.tensor_tensor(out=fc[:], in0=act[:, 128:256],
                                                in1=ct[:], op=ALU.mult)
                    cn = outp.tile([128, 128], f16, tag="cn")
                    nc.vector.tensor_tensor(out=cn[:], in0=fc[:], in1=it[:],
                                            op=ALU.add)
                    tct = outp.tile([128, 128], f16, tag="tct")
                    nc.scalar.activation(out=tct[:], in_=cn[:], func=AFT)
                    hn = outp.tile([128, 128], f16, tag="hn")
                    nc.vector.tensor_tensor(out=hn[:], in0=act[:, 256:384],
                                            in1=tct[:], op=ALU.mult)
                    nc.sync.dma_start(out=OUT[t * 128:(t + 1) * 128, :],
                                      in_=hn[:])

            if variant == "full":
                if reps == 1:
                    phase_a()
                    phase_b()
                    phase_c()
                else:
                    for _ in range(reps):
                        phase_a()
                        phase_b()
                        phase_c()
            elif variant == "a_only":
                tc.For_i_unrolled(0, reps, 1, phase_a, max_unroll=1)
            elif variant == "b_only":
                for _ in range(reps):
                    phase_b()
            elif variant == "ab":
                for _ in range(reps):
                    phase_a()
                    phase_b()
            elif variant == "c_only":
                phase_a()
                phase_b()
                for _ in range(reps):
                    phase_c()
            elif variant.startswith("c_sim"):
                code = variant[5:]
                kw = {}
                if code and code[0].isdigit():
                    kw["lvl"] = int(code[0])
                if "A" in code:
                    kw["acts"] = False
                if "E" in code:
                    kw["ew"] = False
                if "P" in code:
                    kw["pooltt"] = False
                nc.vector.memset(t1T_all[:], 0.0)
                for _ in range(reps):
                    phase_c(src=Hc, **kw)
            else:
                raise ValueError(variant)

    nc.compile()
    # Align SWDGE queue assignment with the DMASW semaphore lanes the tile
    # scheduler handed out (scheduled order, mod 8). queue = lane mod 4
    # keeps each lane's semaphore incremented from a single queue, which
    # the runtime sim requires and which balances queues the same way.
    for blk in nc.m.functions[0].blocks:
        for inst in blk.instructions:
            if (type(inst).__name__ == "InstDMAGatherAnt"
                    and inst.bass_scheduled_proc is not None
                    and 11 <= inst.bass_scheduled_proc <= 18):
                inst.queue_num = (inst.bass_scheduled_proc - 11) % 4
    return nc


def kernel(X, edge_index, edge_weight, H, C, W, b, conv_W, conv_b):
    from concourse.bass_utils import run_bass_kernel_spmd

    in_maps, meta = _host_prep(X, edge_index, edge_weight, H, C, W, b,
                               conv_W, conv_b)
    key = (meta["caps"], meta["groups"])
    if key not in _CACHE:
        _CACHE[key] = _build_program(meta)
    nc = _CACHE[key]

    res = run_bass_kernel_spmd(nc, in_maps, list(range(NCORES)))
    out = np.empty((N, D), np.float32)
    for d in range(NCORES):
        out[d * NPC:(d + 1) * NPC] = res.results[d]["OUT"][:NPC].astype(
            np.float32)
    return out

